# revision 1
# baseline (speedup 1.0000x reference)
"""Trainium2 Bass kernel for nn_EnhancedMoEModel (soft-clustered MoE inference).

Model (per row b of x[B,32], E=8 experts, H=64, H2=32):
    h1[e] = relu(x @ W1[e] + b1[e])            # [B,64] per expert
    h2[e] = relu(h1[e] @ W2[e] + b2[e])        # [B,32]
    eo[e] = sigmoid(h2[e] @ W3[e] + b3[e])     # [B,1]
    out[b] = sum_e probs[b,e] * eo[e][b]

Strategy: data-parallel over 8 NeuronCores (B=524288 -> 65536 rows/core).
All matmuls run weight-stationary in the transposed domain (units on
partitions, batch on the free axis, N=512 per block) with the expert dim
packed into partitions: expert pairs for layer 1 (2x64=128 outputs/matmul)
and zero-padded accumulating pair matmuls for layer 2 that put a full
expert quad (4x32=128) in each PSUM bank. Layer 3 accumulates the logits of
a whole 4-block group into one [32,512] PSUM bank (block jj's experts in
rows 8*jj..8*jj+7 via zero-padded stationary columns), so the sigmoid and
the final combine run once per 4 blocks at full free-dim efficiency.

x is transposed on the host (layout choice) with a ones-row appended so b1
rides inside the layer-1 matmul. b2 is applied by the per-partition bias
operand of the PSUM->SBUF relu evacuations (split between ScalarE and
VectorE - the throughput-critical path); b3 by the sigmoid's bias. The
combine transposes sigmoid outputs to row-major via 4 PE transposes per
group and does one mult + one grouped-reduce on VectorE.

Matmul operands use float32r (full fp32 storage, reduced-precision PE mode,
4x faster than fp32 matmul); set MM_DTYPE to float32 for exact-but-slow.
"""

import sys

sys.path.insert(0, "/opt/trn_rl_repo")

import numpy as np

from concourse import bacc, tile
from concourse.bass_utils import run_bass_kernel_spmd
import concourse.mybir as mybir

F32 = mybir.dt.float32
F32R = mybir.dt.float32r
AF = mybir.ActivationFunctionType
ALU = mybir.AluOpType

N_CORES = 8
B_FULL = 524288
D = 32
H = 64
H2 = 32
E = 8
B_SHARD = B_FULL // N_CORES  # 65536
BLK = 512                    # batch rows per block (PSUM bank free size)
GRP = 8                      # blocks per sigmoid/combine group

MM_DTYPE = F32R              # float32r: 1 cyc/row PE; float32: 4 cyc/row
VARIANT = "v0"               # experiment knob used by sweep.py


def build_nc(n_blocks, repeat=1):
    """Build the per-core Bass program for n_blocks 512-row blocks.

    repeat>1 re-processes every block repeat times (same I/O) - used only
    for isolating device compute time from host/RPC overhead when timing.
    """
    nc = bacc.Bacc("TRN2", target_bir_lowering=False, debug=False,
                   num_devices=N_CORES)
    nb = n_blocks
    assert nb % GRP == 0
    rows = nb * BLK

    xT_d = nc.dram_tensor("xT", [D + 1, rows], MM_DTYPE,
                          kind="ExternalInput")
    pr_d = nc.dram_tensor("probs", [rows, E], F32, kind="ExternalInput")
    w1_d = nc.dram_tensor("w1", [D + 1, 512], MM_DTYPE,
                          kind="ExternalInput")
    w2_d = nc.dram_tensor("w2", [128, 512], MM_DTYPE, kind="ExternalInput")
    w3_d = nc.dram_tensor("w3", [128, GRP * 128], MM_DTYPE,
                          kind="ExternalInput")
    b2_d = nc.dram_tensor("b2q", [128, 2], F32, kind="ExternalInput")
    b3_d = nc.dram_tensor("b3g", [GRP * E, 1], F32, kind="ExternalInput")
    id_d = nc.dram_tensor("ident", [GRP * E, GRP * E], F32,
                          kind="ExternalInput")
    out_d = nc.dram_tensor("out", [rows, 1], F32, kind="ExternalOutput")

    v = VARIANT
    big = 0
    with tile.TileContext(nc) as tc:
        with (
            tc.tile_pool(name="const", bufs=1) as cpool,
            tc.tile_pool(name="xin", bufs=3) as xpool,
            tc.tile_pool(name="pin", bufs=2) as ppool,
            tc.tile_pool(name="h1sb", bufs=2) as h1pool,
            tc.tile_pool(name="h2sb", bufs=2) as h2pool,
            tc.tile_pool(name="tail", bufs=2) as tpool,
            tc.tile_pool(name="ps", bufs=1, space="PSUM") as psw,
            tc.tile_pool(name="ps_eo", bufs=1, space="PSUM") as pse,
            tc.tile_pool(name="ps_eT", bufs=1, space="PSUM") as psT,
        ):
            w1 = cpool.tile([D + 1, 512], MM_DTYPE, tag="w1")
            w2 = cpool.tile([128, 512], MM_DTYPE, tag="w2")
            w3 = cpool.tile([128, GRP * 128], MM_DTYPE, tag="w3")
            b2 = cpool.tile([128, 2], F32, tag="b2")
            b3 = cpool.tile([GRP * E, 1], F32, tag="b3")
            id32 = cpool.tile([GRP * E, GRP * E], F32, tag="id32")
            nc.sync.dma_start(out=w1[:], in_=w1_d[:])
            nc.sync.dma_start(out=w2[:], in_=w2_d[:])
            nc.sync.dma_start(out=w3[:], in_=w3_d[:])
            nc.sync.dma_start(out=b2[:], in_=b2_d[:])
            nc.sync.dma_start(out=b3[:], in_=b3_d[:])
            nc.sync.dma_start(out=id32[:], in_=id_d[:])

            for _rep in range(repeat):
                for g in range(nb // GRP):
                    b0g = g * GRP * BLK
                    eo_ps = pse.tile([GRP * E, 512], F32, tag="eo")
                    pbg = ppool.tile([128, 4, GRP, E], F32, tag="pb")

                    pr4 = pr_d[b0g:b0g + GRP * BLK, :].rearrange(
                        "(jj c p) e -> c p jj e", c=4, p=128)
                    for c in range(4):
                        nc.sync.dma_start(out=pbg[:, c, :, :],
                                          in_=pr4[c:c + 1])
                    for jj in range(GRP):
                        b0 = b0g + jj * BLK
                        if jj % 2 == 0:
                            xt2 = xpool.tile([D + 1, 2 * BLK], MM_DTYPE,
                                             tag="xt")
                            nc.sync.dma_start(out=xt2[:],
                                              in_=xT_d[:, b0:b0 + 2 * BLK])
                        xt = xt2[:, BLK * (jj % 2):BLK * (jj % 2 + 1)]

                        # layer 1: x @ W1 (+b1 via ones-row), expert pairs.
                        # Pairs 0,1 stream from PE rows 0-32 and pairs 2,3
                        # from rows 64-96 (a second copy of xT) so the two
                        # pairs of matmuls run on disjoint sub-array rows.
                        h1A_ps = psw.tile([128, 1024], F32, tag="h1A")
                        h1D_ps = psw.tile([128, 1024], F32, tag="h1D")
                        for p in range(4):
                            dst = h1A_ps if p < 2 else h1D_ps
                            col = 512 * (p % 2)
                            nc.tensor.matmul(dst[:, col:col + 512],
                                             w1[:, 128 * p:128 * (p + 1)],
                                             xt, start=True, stop=True)
                        h1A = h1pool.tile([128, 1024], MM_DTYPE, tag="h1A")
                        h1D = h1pool.tile([128, 1024], MM_DTYPE, tag="h1D")
                        nc.scalar.activation(h1A[:], h1A_ps[:], AF.Relu)
                        if v == "split1":
                            nc.vector.tensor_scalar_max(h1D[:], h1D_ps[:],
                                                        0.0)
                        else:
                            nc.scalar.activation(h1D[:], h1D_ps[:], AF.Relu)

                        # layer 2: zero-padded accumulating pair matmuls ->
                        # full expert quad (4x32=128) per [128,512] bank
                        h2A_ps = psw.tile([128, 512], F32, tag="h2A")
                        h2B_ps = psw.tile([128, 512], F32, tag="h2B")
                        nc.tensor.matmul(h2A_ps[:], w2[:, 0:128],
                                         h1A[:, 0:512], start=True, stop=False)
                        nc.tensor.matmul(h2A_ps[:], w2[:, 128:256],
                                         h1A[:, 512:1024], start=False,
                                         stop=True)
                        nc.tensor.matmul(h2B_ps[:], w2[:, 256:384],
                                         h1D[:, 0:512], start=True, stop=False)
                        nc.tensor.matmul(h2B_ps[:], w2[:, 384:512],
                                         h1D[:, 512:1024], start=False,
                                         stop=True)
                        h2A = h2pool.tile([128, 512], MM_DTYPE, tag="h2A")
                        h2B = h2pool.tile([128, 512], MM_DTYPE, tag="h2B")
                        nc.scalar.activation(h2A[:], h2A_ps[:], AF.Relu,
                                             bias=b2[:, 0:1])
                        if v == "act2":
                            nc.scalar.activation(h2B[:], h2B_ps[:], AF.Relu,
                                                 bias=b2[:, 1:2])
                        else:
                            nc.vector.tensor_scalar(h2B[:], h2B_ps[:],
                                                    b2[:, 1:2], 0.0,
                                                    op0=ALU.add, op1=ALU.max)

                        # layer 3: zero-padded quad matmuls accumulate the
                        # whole group's logits into eo rows 8*jj+e
                        nc.tensor.matmul(eo_ps[:],
                                         w3[:, 128 * jj:128 * jj + 64],
                                         h2A[:], start=(jj == 0), stop=False)
                        nc.tensor.matmul(eo_ps[:],
                                         w3[:, 128 * jj + 64:128 * (jj + 1)],
                                         h2B[:], start=False,
                                         stop=(jj == GRP - 1))

                    # per-group tail: sigmoid, transpose, combine, store
                    sig = tpool.tile([GRP * E, 512], F32, tag="sig")
                    nc.scalar.activation(sig[:], eo_ps[:], AF.Sigmoid,
                                         bias=b3[:, 0:1])
                    eT_ps = psT.tile([128, 4, GRP * E], F32, tag="eT")
                    for c in range(4):
                        nc.tensor.transpose(eT_ps[:, c, :],
                                            sig[:, c * 128:(c + 1) * 128],
                                            id32[:])
                    prod = tpool.tile([128, 4, GRP, E], F32, tag="prod")
                    nc.vector.tensor_tensor(
                        prod[:],
                        eT_ps[:].rearrange("p c (jj e) -> p c jj e", e=E),
                        pbg[:], op=ALU.mult)
                    res = tpool.tile([128, 4 * GRP], F32, tag="res")
                    nc.vector.tensor_reduce(
                        res[:].rearrange("p (c jj) -> p c jj", jj=GRP),
                        prod[:], axis=mybir.AxisListType.X, op=ALU.add)
                    out_v = out_d[b0g:b0g + GRP * BLK, :].rearrange(
                        "(jj cp) o -> cp jj o", jj=GRP)
                    for c in range(4):
                        nc.gpsimd.dma_start(
                            out=out_v[128 * c:128 * (c + 1)],
                            in_=res[:, GRP * c:GRP * (c + 1)])

    nc.compile()
    return nc


def prep_weights(W1, b1, W2, b2, W3, b3):
    """Host-side packing of the tiny expert weights."""
    f = np.float32
    w1 = np.zeros((D + 1, 512), f)
    for e in range(E):
        w1[:D, 64 * e:64 * (e + 1)] = W1[e]
        w1[D, 64 * e:64 * (e + 1)] = b1[e]
    # w2: four [128,128] zero-padded stationaries. Block q*2+h (quad q,
    # h1-half h) maps pair (2q+h)'s experts into out rows 64h..64h+63.
    w2 = np.zeros((128, 512), f)
    for q in range(2):
        for h in range(2):
            blk = w2[:, 256 * q + 128 * h:256 * q + 128 * (h + 1)]
            for s in range(2):
                e = 4 * q + 2 * h + s
                blk[64 * s:64 * (s + 1),
                    64 * h + 32 * s:64 * h + 32 * (s + 1)] = W2[e]
    # w3: 2*GRP zero-padded [128, GRP*8] stationaries - (block jj, quad q)
    # puts quad q's experts at out rows 8*jj + 4*q + ee.
    w3 = np.zeros((128, GRP * 128), f)
    for jj in range(GRP):
        for q in range(2):
            blk = w3[:, 128 * jj + 64 * q:128 * jj + 64 * (q + 1)]
            for ee in range(4):
                blk[32 * ee:32 * (ee + 1), 8 * jj + 4 * q + ee] = \
                    W3[4 * q + ee, :, 0]
    b2q = np.zeros((128, 2), f)
    for q in range(2):
        for ee in range(4):
            b2q[32 * ee:32 * (ee + 1), q] = b2[4 * q + ee]
    b3g = np.tile(b3.reshape(E), GRP).reshape(GRP * E, 1).astype(f)
    ident = np.eye(GRP * E, dtype=f)
    return {"w1": w1, "w2": w2, "w3": w3, "b2q": b2q, "b3g": b3g,
            "ident": ident}


_NC_CACHE = {}


def _get_nc(n_blocks, repeat=1):
    key = (n_blocks, repeat)
    if key not in _NC_CACHE:
        _NC_CACHE[key] = build_nc(n_blocks, repeat)
    return _NC_CACHE[key]


def kernel(x, soft_cluster_probs, W1, b1, W2, b2, W3, b3, _trace=False):
    x = np.asarray(x, np.float32)
    probs = np.asarray(soft_cluster_probs, np.float32)
    B = x.shape[0]
    assert B % N_CORES == 0
    shard = B // N_CORES
    assert shard % (GRP * BLK) == 0
    nb = shard // BLK

    wmap = prep_weights(np.asarray(W1, np.float32), np.asarray(b1, np.float32),
                        np.asarray(W2, np.float32), np.asarray(b2, np.float32),
                        np.asarray(W3, np.float32), np.asarray(b3, np.float32))

    xT = np.empty((D + 1, B), np.float32)
    xT[:D] = x.T
    xT[D] = 1.0

    in_maps = []
    for c in range(N_CORES):
        sl = slice(c * shard, (c + 1) * shard)
        m = {"xT": np.ascontiguousarray(xT[:, sl]),
             "probs": np.ascontiguousarray(probs[sl])}
        m.update(wmap)
        in_maps.append(m)

    nc = _get_nc(nb)
    kw = {}
    if _trace:
        kw = dict(trace=True)
    res = run_bass_kernel_spmd(nc, in_maps, core_ids=list(range(N_CORES)),
                               **kw)
    out = np.concatenate([res.results[c]["out"] for c in range(N_CORES)],
                         axis=0)
    kernel.last_exec_time_ns = res.exec_time_ns
    kernel.last_results = res
    return out


kernel.last_exec_time_ns = None
kernel.last_results = None

